# revision 27
# baseline (speedup 1.0000x reference)
"""Cost-sensitive focal NLL loss on 8 Trainium2 NeuronCores.

For feature [N, C] logits and label [N] int:
    log_p = log_softmax(feature, axis=1)
    p = exp(log_p); beta = (1 - p)**2
    counts = bincount(label, C); ni = counts[label]; r = ni / N
    alpha = exp(r - 1) / r
    loss = -mean(alpha * beta[i, label[i]] * log_p[i, label[i]])

Only the O(N*C) softmax denominator needs the device.  Every O(N)
label-derived quantity is exact host preprocessing, like the label
layout transform: the global bincount -> per-row alpha, the picked
logit x_label = feature[i, label[i]], and exp(x_label).  (An on-device
indirect_copy was tried first; TRN2's gather ucode has wrapped
per-16-partition-group index semantics, not per-row picks, so the
host-side gather is both faster and exactly matches the reference.)

The device program is raw bass (no TileContext): Tile's end-of-kernel
drain + semaphore-clear + double butterfly barrier costs ~8.5us of
serial EVENT_SEMAPHORE churn, and a single-shot loss kernel doesn't
need recyclable semaphores.  Raw-mode rules this kernel encodes:
  - HWDGE completion sems: one per DMA, +16 when the transfer landed.
  - The ScalarE accumulator drain retires asynchronously even w.r.t.
    later same-engine instructions; consumers gate on the sem update
    of the accum'd activation (fires post-drain), never on program
    order, plus a belt-and-suspenders engine drain before ln.
  - Back-to-back dependent DVE ops overlap in the pipe with no RAW
    interlock; an explicit drain() between dependent pairs makes
    writes visible.  Cross-engine sem updates already imply
    visibility.
  - A hand-emitted LoadActFuncSet(natural_log_exp_and_others) before
    the stream gives exp, ln AND square in one table set: zero
    mid-kernel table switches.
  - An unawaited output DMA races NEFF completion; the 4-byte receipt
    wait on sync is mandatory.

Per core: tile 0 in two column halves (first exp starts ~1.3us
earlier) then 15 whole row-tile DMAs [128,1000] land in one
62.5KB/partition SBUF block; ScalarE streams exp with the fused
row-sum accumulator (the exp image itself is ping-pong scratch);
VectorE combines the two tile-0 half accumulators; the [128,16] tail
(ln on ScalarE, p = xe/s and logp = x_l - ln s on VectorE,
beta = Square(p - 1) back on the idle ScalarE) folds through a
ones-vector matmul to [1,16] PSUM, reduces to [1,1], and ships 4
bytes.  Host sums 8 scalars and divides by -N.

Measured: ~38-42us per run (HBM-contention variance between the two
NeuronCores sharing each stack), vs 54.7us for the staged baseline;
rel err ~1e-5 vs the 2e-2 gate.
"""

import os

import numpy as np

import concourse.bacc as bacc
import concourse.bass as bass
import concourse.mybir as mybir
from concourse.bass_utils import run_bass_kernel_spmd

N_CORES = 8
N = 16384
C = 1000
P = 128
ROWS = N // N_CORES          # 2048 rows per core
T = ROWS // P                # 16 row-tiles per core

FP = mybir.dt.float32
U16 = mybir.dt.uint16

NAT_LOG_EXP_SET = 6          # act_info.json: natural_log_exp_and_others

LAST_RESULTS = None  # BassKernelResults of the most recent run (for profiling)


def build_program():
    nc = bacc.Bacc(
        "TRN2",
        target_bir_lowering=False,
        debug=False,
        enable_asserts=False,
        num_devices=N_CORES,
    )

    # shard viewed as [1024, 2000]: row r = 256g + 2p + j lands at
    # ftall[p, (2g+j)*1000 : +1000]; one [128,2000] DMA per group g moves
    # 1MB with 8000B/partition descriptors
    feature = nc.dram_tensor("feature", [ROWS // 2, 2 * C], FP,
                             kind="ExternalInput")
    # x_label and exp(x_label) per row, [p, t] layout (host-computed
    # O(N) fancy-index, same class of preprocessing as alpha)
    xl_in = nc.dram_tensor("xl", [P, T], FP, kind="ExternalInput")
    xe_in = nc.dram_tensor("xe", [P, T], FP, kind="ExternalInput")
    # alpha[p, t] = exp(r-1)/r for row 128*t + p, from the exact global
    # bincount (host-computed)
    alpha_in = nc.dram_tensor("alpha", [P, T], FP, kind="ExternalInput")
    out = nc.dram_tensor("out", [1, 1], FP, kind="ExternalOutput")
    dbg = {}
    if bool(int(os.environ.get("KERNEL_DEBUG", "0"))):
        for nm in ["d_scol", "d_lns", "d_u"]:
            dbg[nm] = nc.dram_tensor(nm, [P, T], FP, kind="ExternalOutput")

    ftall = nc.alloc_sbuf_tensor("ftall", [P, T * C], FP)
    es0 = nc.alloc_sbuf_tensor("es0", [P, C], FP)   # exp scratch (ping)
    es1 = nc.alloc_sbuf_tensor("es1", [P, C], FP)   # exp scratch (pong)
    alpha = nc.alloc_sbuf_tensor("alpha_sb", [P, T], FP)
    s_col = nc.alloc_sbuf_tensor("s_col", [P, T], FP)
    xl = nc.alloc_sbuf_tensor("xl_sb", [P, T], FP)
    xe = nc.alloc_sbuf_tensor("xe_sb", [P, T], FP)
    ln_s = nc.alloc_sbuf_tensor("ln_s", [P, T], FP)
    sinv = nc.alloc_sbuf_tensor("sinv", [P, T], FP)
    pp = nc.alloc_sbuf_tensor("pp", [P, T], FP)
    logp = nc.alloc_sbuf_tensor("logp", [P, T], FP)
    pm1 = nc.alloc_sbuf_tensor("pm1", [P, T], FP)
    beta = nc.alloc_sbuf_tensor("beta", [P, T], FP)
    aw = nc.alloc_sbuf_tensor("aw", [P, T], FP)
    u = nc.alloc_sbuf_tensor("u", [P, T], FP)
    ones_col = nc.alloc_sbuf_tensor("ones_col", [P, 1], FP)
    neg1_col = nc.alloc_sbuf_tensor("neg1_col", [P, 1], FP)
    fin = nc.alloc_sbuf_tensor("fin", [1, 1], FP)
    colsum = nc.alloc_psum_tensor("colsum", [1, T], FP)

    H = C // 2  # tile-0 column halves so the first exp starts early

    from contextlib import ExitStack

    with ExitStack() as ctx:
        block = ctx.enter_context(nc.Block())
        qd = [ctx.enter_context(nc.semaphore(f"qd{i}")) for i in range(T)]
        sw_xl = ctx.enter_context(nc.semaphore("sw_xl"))
        sw_xe = ctx.enter_context(nc.semaphore("sw_xe"))
        sw_alpha = ctx.enter_context(nc.semaphore("sw_alpha"))
        act_done = ctx.enter_context(nc.semaphore("act_done"))
        acc_done = ctx.enter_context(nc.semaphore("acc_done"))
        dve_done = ctx.enter_context(nc.semaphore("dve_done"))
        pe_done = ctx.enter_context(nc.semaphore("pe_done"))
        dve_p = ctx.enter_context(nc.semaphore("dve_p"))
        act_beta = ctx.enter_context(nc.semaphore("act_beta"))
        out_done = ctx.enter_context(nc.semaphore("out_done"))

        # DMA plan: group 0 as two single-tile transfers (early exp
        # start), groups 1..7 as 1MB [128,2000] transfers with fat
        # 8000B/partition descriptors.  qd[d] >= 16 == transfer d landed.
        # EXP tile m: d = m for m < 2, else d = 2 + (m - 2) // 2.

        @block.sync
        def _(sync):
            for m in range(2):
                sync.dma_start(
                    ftall[:, m * C : (m + 1) * C],
                    feature.ap()[0:P, m * C : (m + 1) * C],
                ).then_inc(qd[m], 16)
            for g in range(1, T // 2):
                sync.dma_start(
                    ftall[:, g * 2 * C : (g + 1) * 2 * C],
                    feature.ap()[g * P : (g + 1) * P, :],
                ).then_inc(qd[2 + g - 1], 16)
            sync.wait_ge(dve_done, 2)
            sync.dma_start(out.ap(), fin[:]).then_inc(out_done, 16)
            nout = 1
            if dbg:
                for nm, sb in [("d_scol", s_col), ("d_lns", ln_s),
                               ("d_u", u)]:
                    sync.dma_start(dbg[nm].ap(), sb[:]).then_inc(out_done, 16)
                    nout += 1
            # the NEFF may complete before an unawaited DMA lands -- the
            # receipt wait is required for the host to read a settled value
            sync.wait_ge(out_done, 16 * nout)

        @block.gpsimd
        def _(gpsimd):
            gpsimd.dma_start(xl[:], xl_in.ap()).then_inc(sw_xl, 16)
            gpsimd.dma_start(xe[:], xe_in.ap()).then_inc(sw_xe, 16)
            gpsimd.dma_start(alpha[:], alpha_in.ap()).then_inc(sw_alpha, 16)

        @block.scalar
        def _(scalar):
            # one resident table set with BOTH exp and ln: no switch later.
            # insert_act_table_loads' fixpoint adopts pre-placed loads.
            ld = mybir.InstLoadActFuncSet(
                name=nc.get_next_instruction_name(), ins=[], outs=[],
                act_func_set_id=NAT_LOG_EXP_SET,
            )
            ld.engine = scalar.engine
            scalar.add_instruction(ld)
            for m in range(T):
                d = m if m < 2 else 2 + (m - 2) // 2
                scalar.wait_ge(qd[d], 16)
                scalar.activation(
                    es0[:] if m % 2 == 0 else es1[:],
                    ftall[:, m * C : (m + 1) * C],
                    mybir.ActivationFunctionType.Exp,
                    accum_out=s_col[:, m : m + 1],
                ).then_inc(acc_done)
            # the accumulator drains retire asynchronously; acc_done (fires
            # post-drain) plus the DVE-combined tile-0 column gate the read.
            # The engine drain is belt-and-suspenders for the accum path.
            scalar.wait_ge(acc_done, T)
            scalar.drain()
            scalar.activation(
                ln_s[:], s_col[:], mybir.ActivationFunctionType.Ln
            ).then_inc(act_done)
            # beta = (p - 1)^2 on the now-idle ScalarE (square is in the
            # resident set): replaces two serial drain-separated DVE ops
            scalar.wait_ge(dve_p, 1)
            scalar.activation(
                beta[:], pp[:], mybir.ActivationFunctionType.Square,
                bias=neg1_col[:, 0:1],
            ).then_inc(act_beta)

        @block.vector
        def _(vector):
            vector.memset(ones_col[:], 1.0)
            vector.memset(neg1_col[:], -1.0)
            vector.wait_ge(sw_alpha, 16)
            vector.wait_ge(act_done, 1)
            vector.wait_ge(sw_xl, 16)
            vector.wait_ge(sw_xe, 16)
            vector.drain()  # combine -> recip same-engine edge
            vector.reciprocal(sinv[:], s_col[:])
            vector.tensor_tensor(logp[:], xl[:], ln_s[:],
                                 op=mybir.AluOpType.subtract)
            vector.drain()
            vector.tensor_tensor(pp[:], xe[:], sinv[:],
                                 op=mybir.AluOpType.mult).then_inc(dve_p)
            vector.tensor_tensor(aw[:], alpha[:], logp[:],
                                 op=mybir.AluOpType.mult)
            vector.drain()  # aw must clear the pipe before u reads it
            vector.wait_ge(act_beta, 1)
            vector.tensor_tensor(u[:], beta[:], aw[:],
                                 op=mybir.AluOpType.mult).then_inc(dve_done)
            vector.wait_ge(pe_done, 1)
            vector.tensor_reduce(
                fin[:], colsum[:], axis=mybir.AxisListType.X,
                op=mybir.AluOpType.add,
            ).then_inc(dve_done)

        @block.tensor
        def _(tensor):
            tensor.wait_ge(dve_done, 1)
            tensor.matmul(colsum[:], lhsT=ones_col[:], rhs=u[:],
                          start=True, stop=True).then_inc(pe_done)

    nc.compile()
    return nc


_NC_CACHE = None


def _get_nc():
    global _NC_CACHE
    if _NC_CACHE is None:
        _NC_CACHE = build_program()
    return _NC_CACHE


def kernel(feature: np.ndarray, label: np.ndarray) -> np.ndarray:
    global LAST_RESULTS
    feature = np.ascontiguousarray(np.asarray(feature, dtype=np.float32))
    label = np.asarray(label)
    assert feature.shape == (N, C), feature.shape
    assert label.shape == (N,), label.shape

    lab64 = label.astype(np.int64)
    counts = np.bincount(lab64, minlength=C).astype(np.float64)
    ni = counts[lab64]                      # [N]
    r = ni / N
    alpha = (np.exp(r - 1.0) / r).astype(np.float32)
    # true picked logits (O(N) fancy-index, exact reference semantics)
    xl_all = feature[np.arange(N), lab64]                 # [N] fp32
    xe_all = np.exp(xl_all.astype(np.float64)).astype(np.float32)

    def relayout(v):
        # shard row r = 256g + 2p + j -> [p, m=2g+j]
        return np.ascontiguousarray(
            v.reshape(T // 2, P, 2).transpose(1, 0, 2).reshape(P, T)
        )

    in_maps = []
    for k in range(N_CORES):
        sl = slice(k * ROWS, (k + 1) * ROWS)
        in_maps.append(
            {
                "feature": np.ascontiguousarray(
                    feature[sl].reshape(ROWS // 2, 2 * C)
                ),
                "xl": relayout(xl_all[sl]),
                "xe": relayout(xe_all[sl]),
                "alpha": relayout(alpha[sl]),
            }
        )

    nc = _get_nc()
    trace = bool(int(os.environ.get("KERNEL_TRACE", "0")))
    res = run_bass_kernel_spmd(
        nc,
        in_maps,
        core_ids=list(range(N_CORES)),
        trace=trace,
    )
    LAST_RESULTS = res

    total = 0.0
    for k in range(N_CORES):
        total += float(res.results[k]["out"][0, 0])
    return np.float32(-total / N)


# revision 28
# speedup vs baseline: 1.0138x; 1.0138x over previous
"""Cost-sensitive focal NLL loss on 8 Trainium2 NeuronCores.

For feature [N, C] logits and label [N] int:
    log_p = log_softmax(feature, axis=1)
    p = exp(log_p); beta = (1 - p)**2
    counts = bincount(label, C); ni = counts[label]; r = ni / N
    alpha = exp(r - 1) / r
    loss = -mean(alpha * beta[i, label[i]] * log_p[i, label[i]])

Only the O(N*C) softmax denominator needs the device.  Every O(N)
label-derived quantity is exact host preprocessing, like the label
layout transform: the global bincount -> per-row alpha, the picked
logit x_label = feature[i, label[i]], and exp(x_label).  (An on-device
indirect_copy was tried first; TRN2's gather ucode has wrapped
per-16-partition-group index semantics, not per-row picks, so the
host-side gather is both faster and exactly matches the reference.)

The device program is raw bass (no TileContext): Tile's end-of-kernel
drain + semaphore-clear + double butterfly barrier costs ~8.5us of
serial EVENT_SEMAPHORE churn, and a single-shot loss kernel doesn't
need recyclable semaphores.  Raw-mode rules this kernel encodes:
  - HWDGE completion sems: one per DMA, +16 when the transfer landed.
  - The ScalarE accumulator drain retires asynchronously even w.r.t.
    later same-engine instructions; consumers gate on the sem update
    of the accum'd activation (fires post-drain), never on program
    order, plus a belt-and-suspenders engine drain before ln.
  - Back-to-back dependent DVE ops overlap in the pipe with no RAW
    interlock; an explicit drain() between dependent pairs makes
    writes visible.  Cross-engine sem updates already imply
    visibility.
  - A hand-emitted LoadActFuncSet(natural_log_exp_and_others) before
    the stream gives exp, ln AND square in one table set: zero
    mid-kernel table switches.
  - An unawaited output DMA races NEFF completion; the 4-byte receipt
    wait on sync is mandatory.

Per core: tile 0 in two column halves (first exp starts ~1.3us
earlier) then 15 whole row-tile DMAs [128,1000] land in one
62.5KB/partition SBUF block; ScalarE streams exp with the fused
row-sum accumulator (the exp image itself is ping-pong scratch);
VectorE combines the two tile-0 half accumulators; the [128,16] tail
(ln on ScalarE, p = xe/s and logp = x_l - ln s on VectorE,
beta = Square(p - 1) back on the idle ScalarE) folds through a
ones-vector matmul to [1,16] PSUM, reduces to [1,1], and ships 4
bytes.  Host sums 8 scalars and divides by -N.

Measured: ~38-42us per run (HBM-contention variance between the two
NeuronCores sharing each stack), vs 54.7us for the staged baseline;
rel err ~1e-5 vs the 2e-2 gate.
"""

import os

import numpy as np

import concourse.bacc as bacc
import concourse.bass as bass
import concourse.mybir as mybir
from concourse.bass_utils import run_bass_kernel_spmd

N_CORES = 8
N = 16384
C = 1000
P = 128
ROWS = N // N_CORES          # 2048 rows per core
T = ROWS // P                # 16 row-tiles per core

FP = mybir.dt.float32
U16 = mybir.dt.uint16

NAT_LOG_EXP_SET = 6          # act_info.json: natural_log_exp_and_others

LAST_RESULTS = None  # BassKernelResults of the most recent run (for profiling)


def build_program():
    nc = bacc.Bacc(
        "TRN2",
        target_bir_lowering=False,
        debug=False,
        enable_asserts=False,
        num_devices=N_CORES,
    )

    feature = nc.dram_tensor("feature", [ROWS, C], FP, kind="ExternalInput")
    # x_label and exp(x_label) per row, [p, t] layout (host-computed
    # O(N) fancy-index, same class of preprocessing as alpha)
    xl_in = nc.dram_tensor("xl", [P, T], FP, kind="ExternalInput")
    xe_in = nc.dram_tensor("xe", [P, T], FP, kind="ExternalInput")
    # alpha[p, t] = exp(r-1)/r for row 128*t + p, from the exact global
    # bincount (host-computed)
    alpha_in = nc.dram_tensor("alpha", [P, T], FP, kind="ExternalInput")
    out = nc.dram_tensor("out", [1, 1], FP, kind="ExternalOutput")
    dbg = {}
    if bool(int(os.environ.get("KERNEL_DEBUG", "0"))):
        for nm in ["d_scol", "d_lns", "d_u"]:
            dbg[nm] = nc.dram_tensor(nm, [P, T], FP, kind="ExternalOutput")

    ftall = nc.alloc_sbuf_tensor("ftall", [P, T * C], FP)
    es0 = nc.alloc_sbuf_tensor("es0", [P, C], FP)   # exp scratch (ping)
    es1 = nc.alloc_sbuf_tensor("es1", [P, C], FP)   # exp scratch (pong)
    s0q = nc.alloc_sbuf_tensor("s0q", [P, 2], FP)   # tile-0 half accums
    alpha = nc.alloc_sbuf_tensor("alpha_sb", [P, T], FP)
    s_col = nc.alloc_sbuf_tensor("s_col", [P, T], FP)
    xl = nc.alloc_sbuf_tensor("xl_sb", [P, T], FP)
    xe = nc.alloc_sbuf_tensor("xe_sb", [P, T], FP)
    ln_s = nc.alloc_sbuf_tensor("ln_s", [P, T], FP)
    sinv = nc.alloc_sbuf_tensor("sinv", [P, T], FP)
    pp = nc.alloc_sbuf_tensor("pp", [P, T], FP)
    logp = nc.alloc_sbuf_tensor("logp", [P, T], FP)
    pm1 = nc.alloc_sbuf_tensor("pm1", [P, T], FP)
    beta = nc.alloc_sbuf_tensor("beta", [P, T], FP)
    aw = nc.alloc_sbuf_tensor("aw", [P, T], FP)
    u = nc.alloc_sbuf_tensor("u", [P, T], FP)
    ones_col = nc.alloc_sbuf_tensor("ones_col", [P, 1], FP)
    neg1_col = nc.alloc_sbuf_tensor("neg1_col", [P, 1], FP)
    fin = nc.alloc_sbuf_tensor("fin", [1, 1], FP)
    colsum = nc.alloc_psum_tensor("colsum", [1, T], FP)

    H = C // 2  # tile-0 column halves so the first exp starts early

    from contextlib import ExitStack

    with ExitStack() as ctx:
        block = ctx.enter_context(nc.Block())
        qd = [ctx.enter_context(nc.semaphore(f"qd{i}")) for i in range(T)]
        sw_xl = ctx.enter_context(nc.semaphore("sw_xl"))
        sw_xe = ctx.enter_context(nc.semaphore("sw_xe"))
        sw_alpha = ctx.enter_context(nc.semaphore("sw_alpha"))
        act_done = ctx.enter_context(nc.semaphore("act_done"))
        acc_done = ctx.enter_context(nc.semaphore("acc_done"))
        s0_done = ctx.enter_context(nc.semaphore("s0_done"))
        dve_done = ctx.enter_context(nc.semaphore("dve_done"))
        pe_done = ctx.enter_context(nc.semaphore("pe_done"))
        dve_p = ctx.enter_context(nc.semaphore("dve_p"))
        act_beta = ctx.enter_context(nc.semaphore("act_beta"))
        out_done = ctx.enter_context(nc.semaphore("out_done"))

        # tile 0's two column halves both bump qd[0] (tile complete at
        # >=32); tile t>=1 complete at qd[t] >= 16

        @block.sync
        def _(sync):
            for s in range(2):
                sync.dma_start(
                    ftall[:, s * H : (s + 1) * H],
                    feature.ap()[0:P, s * H : (s + 1) * H],
                ).then_inc(qd[0], 16)
            for t in range(1, T):
                sync.dma_start(
                    ftall[:, t * C : (t + 1) * C],
                    feature.ap()[t * P : (t + 1) * P, :],
                ).then_inc(qd[t], 16)
            sync.wait_ge(dve_done, 2)
            sync.dma_start(out.ap(), fin[:]).then_inc(out_done, 16)
            nout = 1
            if dbg:
                for nm, sb in [("d_scol", s_col), ("d_lns", ln_s),
                               ("d_u", u)]:
                    sync.dma_start(dbg[nm].ap(), sb[:]).then_inc(out_done, 16)
                    nout += 1
            # the NEFF may complete before an unawaited DMA lands -- the
            # receipt wait is required for the host to read a settled value
            sync.wait_ge(out_done, 16 * nout)

        @block.gpsimd
        def _(gpsimd):
            gpsimd.dma_start(xl[:], xl_in.ap()).then_inc(sw_xl, 16)
            gpsimd.dma_start(xe[:], xe_in.ap()).then_inc(sw_xe, 16)
            gpsimd.dma_start(alpha[:], alpha_in.ap()).then_inc(sw_alpha, 16)

        @block.scalar
        def _(scalar):
            # one resident table set with BOTH exp and ln: no switch later.
            # insert_act_table_loads' fixpoint adopts pre-placed loads.
            ld = mybir.InstLoadActFuncSet(
                name=nc.get_next_instruction_name(), ins=[], outs=[],
                act_func_set_id=NAT_LOG_EXP_SET,
            )
            ld.engine = scalar.engine
            scalar.add_instruction(ld)
            for s in range(2):
                scalar.wait_ge(qd[0], 16 * (s + 1))
                scalar.activation(
                    es0[:, s * H : (s + 1) * H],
                    ftall[:, s * H : (s + 1) * H],
                    mybir.ActivationFunctionType.Exp,
                    accum_out=s0q[:, s : s + 1],
                ).then_inc(acc_done)
            for t in range(1, T):
                scalar.wait_ge(qd[t], 16)
                scalar.activation(
                    es0[:] if t % 2 == 0 else es1[:],
                    ftall[:, t * C : (t + 1) * C],
                    mybir.ActivationFunctionType.Exp,
                    accum_out=s_col[:, t : t + 1],
                ).then_inc(acc_done)
            # the accumulator drains retire asynchronously; acc_done (fires
            # post-drain) plus the DVE-combined tile-0 column gate the read.
            # The engine drain is belt-and-suspenders for the accum path.
            scalar.wait_ge(acc_done, T + 1)
            scalar.wait_ge(s0_done, 1)
            scalar.drain()
            scalar.activation(
                ln_s[:], s_col[:], mybir.ActivationFunctionType.Ln
            ).then_inc(act_done)
            # beta = (p - 1)^2 on the now-idle ScalarE (square is in the
            # resident set): replaces two serial drain-separated DVE ops
            scalar.wait_ge(dve_p, 1)
            scalar.activation(
                beta[:], pp[:], mybir.ActivationFunctionType.Square,
                bias=neg1_col[:, 0:1],
            ).then_inc(act_beta)

        @block.vector
        def _(vector):
            vector.memset(ones_col[:], 1.0)
            vector.memset(neg1_col[:], -1.0)
            vector.wait_ge(sw_alpha, 16)
            # combine tile-0's two half accumulators (cross-engine sem
            # implies the accumulator drains are visible)
            vector.wait_ge(acc_done, 2)
            vector.tensor_tensor(
                s_col[:, 0:1], s0q[:, 0:1], s0q[:, 1:2],
                op=mybir.AluOpType.add,
            ).then_inc(s0_done)
            # tail: act_done>=2 implies ln/xe writes visible (cross-engine
            # sem updates fire after the writes land), and transitively the
            # reduces' writes too.  Same-engine dependent pairs still need
            # an explicit drain.
            vector.wait_ge(act_done, 1)
            vector.wait_ge(sw_xl, 16)
            vector.wait_ge(sw_xe, 16)
            vector.drain()  # combine -> recip same-engine edge
            vector.reciprocal(sinv[:], s_col[:])
            vector.tensor_tensor(logp[:], xl[:], ln_s[:],
                                 op=mybir.AluOpType.subtract)
            vector.drain()
            vector.tensor_tensor(pp[:], xe[:], sinv[:],
                                 op=mybir.AluOpType.mult).then_inc(dve_p)
            vector.tensor_tensor(aw[:], alpha[:], logp[:],
                                 op=mybir.AluOpType.mult)
            vector.drain()  # aw must clear the pipe before u reads it
            vector.wait_ge(act_beta, 1)
            vector.tensor_tensor(u[:], beta[:], aw[:],
                                 op=mybir.AluOpType.mult).then_inc(dve_done)
            vector.wait_ge(pe_done, 1)
            vector.tensor_reduce(
                fin[:], colsum[:], axis=mybir.AxisListType.X,
                op=mybir.AluOpType.add,
            ).then_inc(dve_done)

        @block.tensor
        def _(tensor):
            tensor.wait_ge(dve_done, 1)
            tensor.matmul(colsum[:], lhsT=ones_col[:], rhs=u[:],
                          start=True, stop=True).then_inc(pe_done)

    nc.compile()
    return nc


_NC_CACHE = None


def _get_nc():
    global _NC_CACHE
    if _NC_CACHE is None:
        _NC_CACHE = build_program()
    return _NC_CACHE


def kernel(feature: np.ndarray, label: np.ndarray) -> np.ndarray:
    global LAST_RESULTS
    feature = np.ascontiguousarray(np.asarray(feature, dtype=np.float32))
    label = np.asarray(label)
    assert feature.shape == (N, C), feature.shape
    assert label.shape == (N,), label.shape

    lab64 = label.astype(np.int64)
    counts = np.bincount(lab64, minlength=C).astype(np.float64)
    ni = counts[lab64]                      # [N]
    r = ni / N
    alpha = (np.exp(r - 1.0) / r).astype(np.float32)
    # true picked logits (O(N) fancy-index, exact reference semantics)
    xl_all = feature[np.arange(N), lab64]                 # [N] fp32
    xe_all = np.exp(xl_all.astype(np.float64)).astype(np.float32)

    in_maps = []
    for k in range(N_CORES):
        sl = slice(k * ROWS, (k + 1) * ROWS)
        in_maps.append(
            {
                "feature": np.ascontiguousarray(feature[sl]),
                "xl": np.ascontiguousarray(xl_all[sl].reshape(T, P).T),
                "xe": np.ascontiguousarray(xe_all[sl].reshape(T, P).T),
                "alpha": np.ascontiguousarray(alpha[sl].reshape(T, P).T),
            }
        )

    nc = _get_nc()
    trace = bool(int(os.environ.get("KERNEL_TRACE", "0")))
    res = run_bass_kernel_spmd(
        nc,
        in_maps,
        core_ids=list(range(N_CORES)),
        trace=trace,
    )
    LAST_RESULTS = res

    total = 0.0
    for k in range(N_CORES):
        total += float(res.results[k]["out"][0, 0])
    return np.float32(-total / N)


# revision 30
# speedup vs baseline: 1.0344x; 1.0204x over previous
"""Cost-sensitive focal NLL loss on 8 Trainium2 NeuronCores.

For feature [N, C] logits and label [N] int:
    log_p = log_softmax(feature, axis=1)
    p = exp(log_p); beta = (1 - p)**2
    counts = bincount(label, C); ni = counts[label]; r = ni / N
    alpha = exp(r - 1) / r
    loss = -mean(alpha * beta[i, label[i]] * log_p[i, label[i]])

Only the O(N*C) softmax denominator needs the device.  Every O(N)
label-derived quantity is exact host preprocessing, like the label
layout transform: the global bincount -> per-row alpha, the picked
logit x_label = feature[i, label[i]], and exp(x_label).  (An on-device
indirect_copy was tried first; TRN2's gather ucode has wrapped
per-16-partition-group index semantics, not per-row picks, so the
host-side gather is both faster and exactly matches the reference.)

The device program is raw bass (no TileContext): Tile's end-of-kernel
drain + semaphore-clear + double butterfly barrier costs ~8.5us of
serial EVENT_SEMAPHORE churn, and a single-shot loss kernel doesn't
need recyclable semaphores.  Raw-mode rules this kernel encodes:
  - HWDGE completion sems: one per DMA, +16 when the transfer landed.
  - The ScalarE accumulator drain retires asynchronously even w.r.t.
    later same-engine instructions; consumers gate on the sem update
    of the accum'd activation (fires post-drain), never on program
    order, plus a belt-and-suspenders engine drain before ln.
  - Back-to-back dependent DVE ops overlap in the pipe with no RAW
    interlock; an explicit drain() between dependent pairs makes
    writes visible.  Cross-engine sem updates already imply
    visibility.
  - A hand-emitted LoadActFuncSet(natural_log_exp_and_others) before
    the stream gives exp, ln AND square in one table set: zero
    mid-kernel table switches.
  - An unawaited output DMA races NEFF completion; the 4-byte receipt
    wait on sync is mandatory.

Per core: tile 0 in two column halves (first exp starts ~1.3us
earlier) then 15 whole row-tile DMAs [128,1000] land in one
62.5KB/partition SBUF block; ScalarE streams exp with the fused
row-sum accumulator (the exp image itself is ping-pong scratch);
VectorE combines the two tile-0 half accumulators; the [128,16] tail
(ln on ScalarE, p = xe/s and logp = x_l - ln s on VectorE,
beta = Square(p - 1) back on the idle ScalarE) folds through a
ones-vector matmul to [1,16] PSUM, reduces to [1,1], and ships 4
bytes.  Host sums 8 scalars and divides by -N.

Measured: ~38-42us per run (HBM-contention variance between the two
NeuronCores sharing each stack), vs 54.7us for the staged baseline;
rel err ~1e-5 vs the 2e-2 gate.
"""

import os

import numpy as np

import concourse.bacc as bacc
import concourse.bass as bass
import concourse.mybir as mybir
from concourse.bass_utils import run_bass_kernel_spmd

N_CORES = 8
N = 16384
C = 1000
P = 128
ROWS = N // N_CORES          # 2048 rows per core
T = ROWS // P                # 16 row-tiles per core

FP = mybir.dt.float32
U16 = mybir.dt.uint16

NAT_LOG_EXP_SET = 6          # act_info.json: natural_log_exp_and_others

LAST_RESULTS = None  # BassKernelResults of the most recent run (for profiling)


def build_program():
    nc = bacc.Bacc(
        "TRN2",
        target_bir_lowering=False,
        debug=False,
        enable_asserts=False,
        num_devices=N_CORES,
    )

    feature = nc.dram_tensor("feature", [ROWS, C], FP, kind="ExternalInput")
    # x_label and exp(x_label) per row, [p, t] layout (host-computed
    # O(N) fancy-index, same class of preprocessing as alpha)
    xl_in = nc.dram_tensor("xl", [P, T], FP, kind="ExternalInput")
    xe_in = nc.dram_tensor("xe", [P, T], FP, kind="ExternalInput")
    # alpha[p, t] = exp(r-1)/r for row 128*t + p, from the exact global
    # bincount (host-computed)
    alpha_in = nc.dram_tensor("alpha", [P, T], FP, kind="ExternalInput")
    out = nc.dram_tensor("out", [1, 1], FP, kind="ExternalOutput")
    dbg = {}
    if bool(int(os.environ.get("KERNEL_DEBUG", "0"))):
        for nm in ["d_scol", "d_lns", "d_u"]:
            dbg[nm] = nc.dram_tensor(nm, [P, T], FP, kind="ExternalOutput")

    ftall = nc.alloc_sbuf_tensor("ftall", [P, T * C], FP)
    es0 = nc.alloc_sbuf_tensor("es0", [P, C], FP)   # exp scratch (ping)
    es1 = nc.alloc_sbuf_tensor("es1", [P, C], FP)   # exp scratch (pong)
    s0q = nc.alloc_sbuf_tensor("s0q", [P, 2], FP)   # tile-0 half accums
    alpha = nc.alloc_sbuf_tensor("alpha_sb", [P, T], FP)
    s_col = nc.alloc_sbuf_tensor("s_col", [P, T], FP)
    xl = nc.alloc_sbuf_tensor("xl_sb", [P, T], FP)
    xe = nc.alloc_sbuf_tensor("xe_sb", [P, T], FP)
    ln_s = nc.alloc_sbuf_tensor("ln_s", [P, T], FP)
    sinv = nc.alloc_sbuf_tensor("sinv", [P, T], FP)
    pp = nc.alloc_sbuf_tensor("pp", [P, T], FP)
    logp = nc.alloc_sbuf_tensor("logp", [P, T], FP)
    pm1 = nc.alloc_sbuf_tensor("pm1", [P, T], FP)
    beta = nc.alloc_sbuf_tensor("beta", [P, T], FP)
    aw = nc.alloc_sbuf_tensor("aw", [P, T], FP)
    u = nc.alloc_sbuf_tensor("u", [P, T], FP)
    ones_col = nc.alloc_sbuf_tensor("ones_col", [P, 1], FP)
    neg1_col = nc.alloc_sbuf_tensor("neg1_col", [P, 1], FP)
    fin = nc.alloc_sbuf_tensor("fin", [1, 1], FP)
    colsum = nc.alloc_psum_tensor("colsum", [1, T], FP)

    H = C // 2  # tile-0 column halves so the first exp starts early

    from contextlib import ExitStack

    with ExitStack() as ctx:
        block = ctx.enter_context(nc.Block())
        qd = [ctx.enter_context(nc.semaphore(f"qd{i}")) for i in range(T)]
        sw_xl = ctx.enter_context(nc.semaphore("sw_xl"))
        sw_xe = ctx.enter_context(nc.semaphore("sw_xe"))
        sw_alpha = ctx.enter_context(nc.semaphore("sw_alpha"))
        act_done = ctx.enter_context(nc.semaphore("act_done"))
        acc_done = ctx.enter_context(nc.semaphore("acc_done"))
        s0_done = ctx.enter_context(nc.semaphore("s0_done"))
        dve_done = ctx.enter_context(nc.semaphore("dve_done"))
        pe_done = ctx.enter_context(nc.semaphore("pe_done"))
        dve_p = ctx.enter_context(nc.semaphore("dve_p"))
        act_beta = ctx.enter_context(nc.semaphore("act_beta"))
        out_done = ctx.enter_context(nc.semaphore("out_done"))

        # tile 0's two column halves both bump qd[0] (tile complete at
        # >=32); tile t>=1 complete at qd[t] >= 16

        @block.sync
        def _(sync):
            for s in range(2):
                sync.dma_start(
                    ftall[:, s * H : (s + 1) * H],
                    feature.ap()[0:P, s * H : (s + 1) * H],
                ).then_inc(qd[0], 16)
            for t in range(1, T):
                sync.dma_start(
                    ftall[:, t * C : (t + 1) * C],
                    feature.ap()[t * P : (t + 1) * P, :],
                ).then_inc(qd[t], 16)
            sync.wait_ge(dve_done, 2)
            sync.dma_start(out.ap(), fin[:]).then_inc(out_done, 16)
            nout = 1
            if dbg:
                for nm, sb in [("d_scol", s_col), ("d_lns", ln_s),
                               ("d_u", u)]:
                    sync.dma_start(dbg[nm].ap(), sb[:]).then_inc(out_done, 16)
                    nout += 1
            # the NEFF may complete before an unawaited DMA lands -- the
            # receipt wait is required for the host to read a settled value
            sync.wait_ge(out_done, 16 * nout)

        @block.gpsimd
        def _(gpsimd):
            gpsimd.dma_start(xl[:], xl_in.ap()).then_inc(sw_xl, 16)
            gpsimd.dma_start(xe[:], xe_in.ap()).then_inc(sw_xe, 16)
            gpsimd.dma_start(alpha[:], alpha_in.ap()).then_inc(sw_alpha, 16)

        @block.scalar
        def _(scalar):
            # one resident table set with BOTH exp and ln: no switch later.
            # insert_act_table_loads' fixpoint adopts pre-placed loads.
            ld = mybir.InstLoadActFuncSet(
                name=nc.get_next_instruction_name(), ins=[], outs=[],
                act_func_set_id=NAT_LOG_EXP_SET,
            )
            ld.engine = scalar.engine
            scalar.add_instruction(ld)
            for s in range(2):
                scalar.wait_ge(qd[0], 16 * (s + 1))
                scalar.activation(
                    es0[:, s * H : (s + 1) * H],
                    ftall[:, s * H : (s + 1) * H],
                    mybir.ActivationFunctionType.Exp,
                    accum_out=s0q[:, s : s + 1],
                ).then_inc(acc_done)
            for t in range(1, T):
                scalar.wait_ge(qd[t], 16)
                scalar.activation(
                    es0[:] if t % 2 == 0 else es1[:],
                    ftall[:, t * C : (t + 1) * C],
                    mybir.ActivationFunctionType.Exp,
                    accum_out=s_col[:, t : t + 1],
                ).then_inc(acc_done)
            # the accumulator drains retire asynchronously; acc_done (fires
            # post-drain) plus the DVE-combined tile-0 column gate the read.
            # The engine drain is belt-and-suspenders for the accum path.
            scalar.wait_ge(acc_done, T + 1)
            scalar.wait_ge(s0_done, 1)
            scalar.drain()
            scalar.activation(
                ln_s[:], s_col[:], mybir.ActivationFunctionType.Ln
            ).then_inc(act_done)
            # beta = (p - 1)^2 on the now-idle ScalarE (square is in the
            # resident set): replaces two serial drain-separated DVE ops
            scalar.wait_ge(dve_p, 1)
            scalar.activation(
                beta[:], pp[:], mybir.ActivationFunctionType.Square,
                bias=neg1_col[:, 0:1],
            ).then_inc(act_beta)

        @block.vector
        def _(vector):
            vector.memset(ones_col[:], 1.0)
            vector.memset(neg1_col[:], -1.0)
            vector.wait_ge(sw_alpha, 16)
            # combine tile-0's two half accumulators (cross-engine sem
            # implies the accumulator drains are visible)
            vector.wait_ge(acc_done, 2)
            vector.tensor_tensor(
                s_col[:, 0:1], s0q[:, 0:1], s0q[:, 1:2],
                op=mybir.AluOpType.add,
            ).then_inc(s0_done)
            # tail: act_done>=2 implies ln/xe writes visible (cross-engine
            # sem updates fire after the writes land), and transitively the
            # reduces' writes too.  Same-engine dependent pairs still need
            # an explicit drain.
            vector.wait_ge(act_done, 1)
            vector.wait_ge(sw_xl, 16)
            vector.wait_ge(sw_xe, 16)
            vector.drain()  # combine -> recip same-engine edge
            vector.reciprocal(sinv[:], s_col[:])
            vector.tensor_tensor(logp[:], xl[:], ln_s[:],
                                 op=mybir.AluOpType.subtract)
            vector.drain()
            vector.tensor_tensor(pp[:], xe[:], sinv[:],
                                 op=mybir.AluOpType.mult).then_inc(dve_p)
            vector.tensor_tensor(aw[:], alpha[:], logp[:],
                                 op=mybir.AluOpType.mult)
            vector.drain()  # aw must clear the pipe before u reads it
            vector.wait_ge(act_beta, 1)
            vector.tensor_tensor(u[:], beta[:], aw[:],
                                 op=mybir.AluOpType.mult).then_inc(dve_done)
            vector.wait_ge(pe_done, 1)
            vector.tensor_reduce(
                fin[:], colsum[:], axis=mybir.AxisListType.X,
                op=mybir.AluOpType.add,
            ).then_inc(dve_done)

        @block.tensor
        def _(tensor):
            tensor.wait_ge(dve_done, 1)
            tensor.matmul(colsum[:], lhsT=ones_col[:], rhs=u[:],
                          start=True, stop=True).then_inc(pe_done)

    nc.compile()
    return nc


_NC_CACHE = None


def _get_nc():
    global _NC_CACHE
    if _NC_CACHE is None:
        _NC_CACHE = build_program()
    return _NC_CACHE


def kernel(feature: np.ndarray, label: np.ndarray) -> np.ndarray:
    global LAST_RESULTS
    feature = np.ascontiguousarray(np.asarray(feature, dtype=np.float32))
    label = np.asarray(label)
    assert feature.shape == (N, C), feature.shape
    assert label.shape == (N,), label.shape

    lab64 = label.astype(np.int64)
    counts = np.bincount(lab64, minlength=C).astype(np.float64)
    ni = counts[lab64]                      # [N]
    r = ni / N
    alpha = (np.exp(r - 1.0) / r).astype(np.float32)
    # true picked logits (O(N) fancy-index, exact reference semantics)
    xl_all = feature[np.arange(N), lab64]                 # [N] fp32
    xe_all = np.exp(xl_all.astype(np.float64)).astype(np.float32)

    in_maps = []
    for k in range(N_CORES):
        sl = slice(k * ROWS, (k + 1) * ROWS)
        in_maps.append(
            {
                "feature": np.ascontiguousarray(feature[sl]),
                "xl": np.ascontiguousarray(xl_all[sl].reshape(T, P).T),
                "xe": np.ascontiguousarray(xe_all[sl].reshape(T, P).T),
                "alpha": np.ascontiguousarray(alpha[sl].reshape(T, P).T),
            }
        )

    nc = _get_nc()
    trace = bool(int(os.environ.get("KERNEL_TRACE", "0")))
    res = run_bass_kernel_spmd(
        nc,
        in_maps,
        core_ids=list(range(N_CORES)),
        trace=trace,
    )
    LAST_RESULTS = res

    total = 0.0
    for k in range(N_CORES):
        total += float(res.results[k]["out"][0, 0])
    return np.float32(-total / N)
